# revision 80
# baseline (speedup 1.0000x reference)
"""Trainium2 Bass kernel for nn_DecoderLayer_15642270892252 (v2, fp8).

Strategy (8 NeuronCores): 2 data-parallel groups over batch B=2; within each
group, 4-way tensor parallel over the 16 heads (4 per core). Attention runs
entirely in fp8e4m3 with DoubleRow matmuls (2 contraction chunks per
instruction at 2x rate): QKV projections, scores (dh=64 split as 2x32), AV
(flipped to [q, dh] orientation with a ones-augmented V column so softmax
denominators fall out of the same matmul), and O-proj. The FFN stays bf16
(fp8 there costs ~1.9e-2 rel err, over the 2e-2 budget); W1/W2 are
SBUF-resident so the 4 chunked FFN passes don't re-stream them.

The O-proj partial sums reduce-scatter in 4 chunks (one per 512-token
q-block; each rank owns 128 rows per chunk), all issued from gpsimd; each
chunk's LN1 + FFN pipeline is emitted interleaved into the remaining
attention blocks (pump()) so the PE stays busy while Act grinds exp.
LayerNorm uses bn_stats + 2-step Newton rsqrt (no Act table thrash: Act
only ever runs Exp/Copy/Relu, which share one activation table); the
tail chunks' LN chains run on DVE (idle there) and the late blocks'
O-proj drains on Act (exp stream finished) to shorten the RS3->FFN tail.

Scales: weights x64 in fp8, activations x4 (Q,K,V,O), exp scale folds
1/(sqrt(dh)*16). The RS wire is fp8: O-proj drains fold 64/256 so the
partials cross at 64x, and the host pre-scales the residual x by 64 --
LN1 is scale-invariant, so the spine normalizes it away (Newton seed
scaled to match). hch/h2 spine and the output are bf16 (host upcasts to
f32). Weight streams are paced one DMA per attention unit (wload), with
wload_until guards keeping pumped FFN matmuls behind their slices. PSUM
rotation: scores/proj "mm" x2, AV "av" x2, FFN2 "acc" x2 banks.
Chunks 0/3's hT transposes run on the PE (identity matmul) -- c0 to
dodge a scheduler-inserted SP-queue wait on the second collective, c3
because the PE is idle on the tail; block 3's pump budget (30) leaves
c2's FFN2 to grind during RS3. In the bias/affine-free case LN1 only
subtracts the mean: relu(y*z)=y*relu(z) and LN2 is invariant to
per-row positive scale, so the rsqrt is never computed (LN2's Newton
seed absorbs the 64x).
"""

import numpy as np
import ml_dtypes

import concourse.bass as bass
import concourse.mybir as mybir
import concourse.tile as tile
from concourse import bacc
from concourse import bass2jax
from concourse.bass2jax import _bass_exec_p, install_neuronx_cc_hook
from concourse.masks import make_identity

F32 = mybir.dt.float32
BF16 = mybir.dt.bfloat16
F8 = mybir.dt.float8e4
AF = mybir.ActivationFunctionType
DR = mybir.MatmulPerfMode.DoubleRow
ALU = mybir.AluOpType
BF = ml_dtypes.bfloat16
NF8 = ml_dtypes.float8_e4m3

B, L, D, H, DH, DFF = 2, 2048, 1024, 16, 64, 4096
EPS = 1e-6
N_CORES = 8
TP = 4
SHARD = L // TP             # 512 rows per rank
HPC = H // TP               # 4 heads per core
GROUPS = [[0, 1, 2, 3], [4, 5, 6, 7]]
NQT = L // 512              # 4 q-blocks == 4 RS chunks
NKT = L // 128              # 16 k-tiles

SW = 64.0                   # weight scale in fp8
DRAIN_QKV = 1.0 / 16.0      # psum(64*Q) -> 4*Q
SCL_EXP = 0.125 / 16.0      # 1/sqrt(dh) / (4*4)
PART_SCALE = 64.0           # fp8 RS wire scale (LN1 is scale-invariant)
DRAIN_PART = PART_SCALE / 256.0   # psum(4*64*part) -> 64*part (fp8 wire)
Y0_LN1 = 1.0 / PART_SCALE   # Newton rsqrt seeds (~1/sqrt(median var));
Y0_LN2 = 0.865              # LN1 input is PART_SCALE*x + wire partials
CDOF = float(D) / float(D - 1)


def rows_of(r):
    """Global L-rows owned by TP rank r (chunk-major: 128 rows per q-block)."""
    return np.concatenate([np.arange(512 * ci + 128 * r, 512 * ci + 128 * (r + 1))
                           for ci in range(NQT)])


def _build(with_bias, with_affine):
    nc = bacc.Bacc()

    xT8 = nc.dram_tensor("xT8", [D, L], F8, kind="ExternalInput")
    xsb = nc.dram_tensor("xsb", [SHARD, D], BF16, kind="ExternalInput")
    wqkv8 = nc.dram_tensor("wqkv8", [128, 8, 6, 128], F8, kind="ExternalInput")
    wo8 = nc.dram_tensor("wo8", [128, 2, D], F8, kind="ExternalInput")
    w1p = nc.dram_tensor("w1p", [32, 128, D], BF16, kind="ExternalInput")
    w2p = nc.dram_tensor("w2p", [32, 128, D], BF16, kind="ExternalInput")
    maskt = nc.dram_tensor("maskt", [128, 128], BF16, kind="ExternalInput")
    if with_bias:
        bqk = nc.dram_tensor("bqk", [128, 4], F32, kind="ExternalInput")
        bv = nc.dram_tensor("bv", [256], F32, kind="ExternalInput")
        b1s = nc.dram_tensor("b1s", [128, 32], F32, kind="ExternalInput")
        bo_b2 = nc.dram_tensor("bo_b2", [2, D], F32, kind="ExternalInput")
    if with_affine:
        lnab = nc.dram_tensor("lnab", [4, D], F32, kind="ExternalInput")

    out = nc.dram_tensor("out", [SHARD, D], BF16, kind="ExternalOutput")

    part = [nc.dram_tensor(f"part{ci}", [512, D], F8) for ci in range(NQT)]
    rs = [nc.dram_tensor(f"rs{ci}", [128, D], F8) for ci in range(NQT)]

    with tile.TileContext(nc) as tc:
        _emit(nc, tc, locals(), with_bias, with_affine)
    nc.finalize()
    return nc


def _emit(nc, tc, t, with_bias, with_affine):
    xT8, xsb, wqkv8, wo8, w1p, w2p, maskt = (
        t["xT8"], t["xsb"], t["wqkv8"], t["wo8"], t["w1p"], t["w2p"], t["maskt"])
    part, rs, out = t["part"], t["rs"], t["out"]

    with tc.tile_pool(name="persist", bufs=1) as P, \
         tc.tile_pool(name="trans", bufs=2) as TR, \
         tc.tile_pool(name="ps", bufs=1, space="PSUM") as PS:

        # ---------------- resident SBUF ----------------
        wqkv_sb = P.tile([128, 8, 6, 128], F8)
        nc.sync.dma_start(out=wqkv_sb, in_=wqkv8[:, :, :, :])
        xT_sb = P.tile([128, 8, L], F8)
        for k in range(8):
            nc.sync.dma_start(out=xT_sb[:, k, :], in_=xT8[k * 128:(k + 1) * 128, :])
        wo_sb = P.tile([128, 2, D], F8)
        nc.sync.dma_start(out=wo_sb, in_=wo8[:, :, :])
        maskt_sb = P.tile([128, 128], BF16)
        nc.sync.dma_start(out=maskt_sb, in_=maskt[:, :])

        qT8 = P.tile([128, 2, L], F8)       # partition = 32*h + dlow
        kT8 = P.tile([128, 2, L], F8)
        v8 = P.tile([128, NKT, HPC, 65], F8)
        w1_sb = P.tile([128, 32, D], BF16)
        w2_sb = P.tile([128, 32, D], BF16)

        identf = P.tile([128, 128], F32)
        make_identity(nc, identf)
        identb = P.tile([128, 128], BF16)
        nc.vector.tensor_copy(out=identb, in_=identf)
        nc.gpsimd.memset(v8[:, :, :, 64:65], 1.0)   # ones-augmented V column

        if with_bias:
            bqk_sb = P.tile([128, 4], F32)
            nc.sync.dma_start(out=bqk_sb, in_=t["bqk"][:, :])
            bv_sb = P.tile([128, HPC, 64], F32)
            nc.sync.dma_start(out=bv_sb,
                              in_=t["bv"][None, :].partition_broadcast(128))
            b1_sb = P.tile([128, 32], F32)
            nc.sync.dma_start(out=b1_sb, in_=t["b1s"][:, :])
            bo_sb = P.tile([128, D], F32)
            nc.sync.dma_start(out=bo_sb, in_=t["bo_b2"][0].partition_broadcast(128))
            b2_sb = P.tile([128, D], F32)
            nc.sync.dma_start(out=b2_sb, in_=t["bo_b2"][1].partition_broadcast(128))
        if with_affine:
            ln_sb = P.tile([128, 4, D], F32)
            nc.sync.dma_start(out=ln_sb,
                              in_=t["lnab"][None, :, :].partition_broadcast(128))

        # W1/W2 resident loads are staggered between attention blocks (SP)
        # so they never starve the part-staging DMAs; see wload() calls below.
        wload_plan = ([("w1", mk) for mk in range(32)]
                      + [("w2", mk) for mk in range(32)])

        def wload(k):
            for _ in range(k):
                if not wload_plan:
                    return
                which, mk = wload_plan.pop(0)
                dst = w1_sb if which == "w1" else w2_sb
                src = w1p if which == "w1" else w2p
                nc.sync.dma_start(out=dst[:, mk, :], in_=src[mk])

        def wload_until(n):
            wload(n - (64 - len(wload_plan)))

        # ---------------- FFN chunk pipeline (pumped closures) ----------------
        boxes = [dict() for _ in range(NQT)]

        def ffn_units(ci):
            bx = boxes[ci]
            units_f1 = []
            units = units_f1

            def u_lnprep():
                # Act-issued: pump pacing places these after the rs chunk has
                # landed so Act doesn't in-order-stall its exp queue.
                rt = TR.tile([128, D], F8, tag="rt", bufs=1, name=f"rt{ci}")
                nc.scalar.dma_start(out=rt, in_=rs[ci][:, :])
                xst = TR.tile([128, D], BF16, tag="xst", bufs=1, name=f"xst{ci}")
                nc.scalar.dma_start(out=xst, in_=xsb[ci * 128:(ci + 1) * 128, :])
                hch = P.tile([128, D], BF16, tag="hch", bufs=2, name=f"hch{ci}")
                bx["h"] = hch
                nc.vector.tensor_add(out=hch, in0=rt, in1=xst)
                if with_bias:
                    nc.vector.tensor_add(out=hch, in0=hch, in1=bo_sb)
                eng = nc.gpsimd if ci in (2,) else nc.vector
                _layernorm(nc, TR, hch, Y0_LN1, ci,
                           ln_sb[:, 0, :] if with_affine else None,
                           ln_sb[:, 1, :] if with_affine else None, eng=eng,
                           apply_scale=with_bias or with_affine)
                if ci in (0, 3):
                    # PE-transpose chunks read hch directly (engine reads are
                    # sem-tracked; the DMA-transpose path needs the snapshot)
                    bx["hb"] = hch
                else:
                    hb = TR.tile([128, D], BF16, tag="hb", bufs=1,
                                 name=f"hb{ci}")
                    eng.tensor_copy(out=hb, in_=hch)
                    bx["hb"] = hb
                bx["hT"] = TR.tile([128, 8, 128], BF16, tag="hT", bufs=1, name=f"hT{ci}")
            units.append(u_lnprep)

            for j0 in range(0, 8, 4):
                def u_tr(j0=j0):
                    hb, hT = bx["hb"], bx["hT"]
                    if ci in (0, 3):
                        # PE transpose: dodges an SP-queue scheduler wait that
                        # parks chunk 0's DMA transposes behind RS1.
                        for j in range(j0, j0 + 4):
                            pt = PS.tile([128, 128], BF16, tag="mm", bufs=2,
                                         name=f"htp{ci}_{j}")
                            nc.tensor.transpose(
                                pt[:, :], hb[:, j * 128:(j + 1) * 128], identb)
                            nc.vector.tensor_copy(out=hT[:, j, :], in_=pt)
                    else:
                        for j in range(j0, j0 + 4):
                            nc.sync.dma_start_transpose(
                                out=hT[:, j, :], in_=hb[:, j * 128:(j + 1) * 128])
                units.append(u_tr)

            def u_f1alloc():
                bx["f1"] = P.tile([128, 32, 128], BF16, tag="f1", bufs=2,
                                  name=f"f1_{ci}")
            units.append(u_f1alloc)

            for mq in range(8):
                def u_f1(mq=mq):
                    wload_until(4 * (mq + 1))
                    hT, f1 = bx["hT"], bx["f1"]
                    psf = PS.tile([128, 4, 128], F32, tag="mm", bufs=2,
                                  name=f"psf{ci}_{mq}")
                    for mm in range(4):
                        m = mq * 4 + mm
                        for k in range(8):
                            nc.tensor.matmul(
                                psf[:, mm, :], w1_sb[:, m, k * 128:(k + 1) * 128],
                                hT[:, k, :], start=(k == 0), stop=(k == 7))
                    if with_bias:
                        for mm in range(4):
                            m = mq * 4 + mm
                            nc.vector.tensor_scalar(
                                out=f1[:, m, :], in0=psf[:, mm, :],
                                scalar1=b1_sb[:, m:m + 1], scalar2=0.0,
                                op0=ALU.add, op1=ALU.max)
                    elif mq % 2:
                        nc.vector.tensor_scalar_max(
                            out=f1[:, mq * 4:(mq + 1) * 4, :], in0=psf,
                            scalar1=0.0)
                    else:
                        nc.scalar.activation(
                            out=f1[:, mq * 4:(mq + 1) * 4, :], in_=psf,
                            func=AF.Relu)
                units.append(u_f1)

            units_f2 = []
            units = units_f2

            for n in range(2):
                for kq in range(8):
                    def u_f2(n=n, kq=kq):
                        wload_until(32 + 4 * (kq + 1))
                        if kq == 0:
                            bx[f"acc{n}"] = PS.tile([128, 512], F32, tag="acc", bufs=2,
                                                    name=f"acc{ci}_{n}")
                        acc, f1 = bx[f"acc{n}"], bx["f1"]
                        for kk in range(4):
                            k = kq * 4 + kk
                            nc.tensor.matmul(
                                acc[:, :], f1[:, k, :],
                                w2_sb[:, k, n * 512:(n + 1) * 512],
                                start=(k == 0), stop=(k == 31))
                    units.append(u_f2)

                def u_f2drain(n=n):
                    acc, hch = bx[f"acc{n}"], bx["h"]
                    nc.vector.tensor_add(out=hch[:, n * 512:(n + 1) * 512],
                                         in0=acc,
                                         in1=hch[:, n * 512:(n + 1) * 512])
                units.append(u_f2drain)

            def u_out():
                hch = bx["h"]
                if with_bias:
                    nc.vector.tensor_add(out=hch, in0=hch, in1=b2_sb)
                y2 = Y0_LN2 if (with_bias or with_affine) \
                    else Y0_LN2 / PART_SCALE
                _layernorm(nc, TR, hch, y2, 4 + ci,
                           ln_sb[:, 2, :] if with_affine else None,
                           ln_sb[:, 3, :] if with_affine else None,
                           eng=nc.gpsimd if ci in (1, 2) else nc.vector)
                nc.sync.dma_start(out=out[ci * 128:(ci + 1) * 128, :], in_=hch)
            units.append(u_out)
            return units_f1, units_f2

        pending = []
        pump_box = {"budget": 10 ** 9}

        def pump(k):
            for _ in range(k):
                if not pending or pump_box["budget"] <= 0:
                    return
                pump_box["budget"] -= 1
                pending.pop(0)()

        # ---------------- QKV ----------------
        def emit_qk(n):
            for i, dst in ((0, qT8), (2, kT8)):
                ps = PS.tile([128, 2, 512], F32, tag="mm", bufs=2,
                             name=f"qk{n}_{i}")
                for a in range(2):
                    for pi in range(4):
                        nc.tensor.matmul(
                            ps[:, a, :], wqkv_sb[:, 2 * pi:2 * pi + 2, i + a, :],
                            xT_sb[:, 2 * pi:2 * pi + 2, n * 512:(n + 1) * 512],
                            start=(pi == 0), stop=(pi == 3), perf_mode=DR)
                od = dst[:, :, n * 512:(n + 1) * 512]
                if with_bias:
                    for a in range(2):
                        nc.vector.tensor_scalar(
                            out=od[:, a, :], in0=ps[:, a, :], scalar1=DRAIN_QKV,
                            scalar2=bqk_sb[:, i + a:i + a + 1],
                            op0=ALU.mult, op1=ALU.add)
                else:
                    nc.vector.tensor_scalar_mul(out=od, in0=ps,
                                                scalar1=DRAIN_QKV)

        def emit_v(t0):
            ps = PS.tile([128, 2, 256], F32, tag="mm", bufs=2, name=f"vv{t0}")
            for tt in range(2):
                for pi in range(4):
                    nc.tensor.matmul(
                        ps[:, tt, :],
                        xT_sb[:, 2 * pi:2 * pi + 2,
                              (t0 + tt) * 128:(t0 + tt + 1) * 128],
                        wqkv_sb[:, 2 * pi:2 * pi + 2, 4:6, :],
                        start=(pi == 0), stop=(pi == 3), perf_mode=DR)
            dst = v8[:, t0:t0 + 2, :, 0:64]
            src = ps.rearrange("p t (h e) -> p t h e", h=HPC)
            nc.vector.tensor_scalar_mul(out=dst, in0=src, scalar1=DRAIN_QKV)
            if with_bias:
                for tt in range(2):
                    nc.vector.tensor_add(out=v8[:, t0 + tt, :, 0:64],
                                         in0=v8[:, t0 + tt, :, 0:64],
                                         in1=bv_sb)

        # ---------------- attention block ----------------
        def attn_block(qi, pump_every=0, pump_n=1):
            npj = 2 * qi + 2
            unit = 0
            # obf[s][:, h, :] = normalized O for token-subtile s, head h
            obfs = [TR.tile([128, HPC, 64], BF16, tag="obf", bufs=5,
                            name=f"obf{qi}_{s}") for s in range(4)]

            def sc_part(h):
                nonlocal unit
                p8s = []
                for pj in range(npj):
                    straddle = pj >= 2 * qi
                    t0 = 2 * (pj - 2 * qi) if straddle else 0
                    ps = PS.tile([128, 2, 512], F32, tag="mm", bufs=2,
                                 name=f"sc{qi}_{h}_{pj}")
                    p8 = TR.tile([128, 2, 512], F8, tag="p8", bufs=9,
                                 name=f"p8_{qi}_{h}_{pj}")
                    p8s.append(p8)
                    qlo = 128 * t0 if straddle else 0
                    for m in range(2):
                        j = 2 * pj + m
                        nc.tensor.matmul(
                            ps[:, m, qlo:512],
                            kT8[32 * h:32 * h + 32, :, j * 128:(j + 1) * 128],
                            qT8[32 * h:32 * h + 32, :,
                                qi * 512 + qlo:(qi + 1) * 512],
                            start=True, stop=True, perf_mode=DR,
                            tile_position=(32 * h, 0))
                    nc.scalar.activation(out=p8[:, :, qlo:512],
                                         in_=ps[:, :, qlo:512],
                                         func=AF.Exp, scale=SCL_EXP)
                    if straddle:
                        for m in range(2):
                            s = t0 + m
                            nc.vector.tensor_mul(
                                out=p8[:, m, s * 128:(s + 1) * 128],
                                in0=p8[:, m, s * 128:(s + 1) * 128],
                                in1=maskt_sb)
                    unit += 1
                    wload(1)
                    if pump_every and unit % pump_every == 0:
                        pump(pump_n)
                return p8s

            def av_part(h, p8s):
                # AV: s-sequential accumulation groups in a single-bank tile
                av = PS.tile([128, 4, 128], F32, tag="av", bufs=2,
                             name=f"av{qi}_{h}")
                for s in range(4):
                    pjs = [pj for pj in range(npj)
                           if not (pj >= 2 * qi and s < 2 * (pj - 2 * qi))]
                    for idx, pj in enumerate(pjs):
                        straddle = pj >= 2 * qi
                        t0 = 2 * (pj - 2 * qi) if straddle else 0
                        first, last = (idx == 0), (idx == len(pjs) - 1)
                        if straddle and s == t0:
                            nc.tensor.matmul(
                                av[:, s, 0:65],
                                p8s[pj][:, 0, s * 128:(s + 1) * 128],
                                v8[:, 2 * pj, h, :],
                                start=first, stop=last)
                        else:
                            nc.tensor.matmul(
                                av[:, s, 0:65],
                                p8s[pj][:, :, s * 128:(s + 1) * 128],
                                v8[:, 2 * pj:2 * pj + 2, h, :],
                                start=first, stop=last, perf_mode=DR)
                rec = TR.tile([128, 4], F32, tag="rec", bufs=2,
                              name=f"rec{qi}_{h}")
                nc.vector.reciprocal(out=rec, in_=av[:, :, 64])
                for s in range(4):
                    nc.scalar.activation(
                        out=obfs[s][:, h, :], in_=av[:, s, 0:64],
                        func=AF.Copy, scale=rec[:, s:s + 1])
                wload(1)
                if pump_every:
                    pump(pump_n)

            # Head lookahead: emit later heads' score matmuls ahead of earlier
            # heads' AV so the PE never parks on an exp round-trip.  Bounded
            # by the p8 pool (9 bufs): block 0 can run all-scores-first,
            # block 1 one head ahead.
            la = 0
            p8s_by_h = {}
            for h in range(HPC):
                p8s_by_h[h] = sc_part(h)
                if h - la >= 0:
                    av_part(h - la, p8s_by_h.pop(h - la))
            for h in sorted(p8s_by_h):
                av_part(h, p8s_by_h[h])

            # transpose + O-proj + stage, per token-subtile
            for s in range(4):
                obf = obfs[s]
                oT = TR.tile([128, 2, 128], F8, tag="oT", name=f"oT{qi}_{s}")
                for hp in range(2):
                    pt = PS.tile([128, 128], BF16, tag="mm", bufs=2,
                                 name=f"otp{qi}_{s}_{hp}")
                    nc.tensor.transpose(
                        pt[:, :],
                        obf[:, 2 * hp:2 * hp + 2, :].rearrange(
                            "p h e -> p (h e)"), identb)
                    if qi <= 1:
                        nc.scalar.activation(out=oT[:, hp, :], in_=pt,
                                             func=AF.Copy)
                    else:
                        nc.vector.tensor_copy(out=oT[:, hp, :], in_=pt)
                pso = PS.tile([128, 2, 512], F32, tag="mm", bufs=2,
                              name=f"pso{qi}_{s}")
                for n in range(2):
                    nc.tensor.matmul(pso[:, n, :], oT[:, :, :],
                                     wo_sb[:, :, n * 512:(n + 1) * 512],
                                     start=True, stop=True, perf_mode=DR)
                st = TR.tile([128, D], F8, tag="st", bufs=2,
                             name=f"st{qi}_{s}")
                psof = pso.rearrange("p n q -> p (n q)")
                nc.scalar.activation(out=st, in_=psof, func=AF.Copy,
                                     scale=DRAIN_PART)
                nc.sync.dma_start(
                    out=part[qi][s * 128:(s + 1) * 128, :], in_=st)
                wload(1)
                if pump_every:
                    pump(pump_n)
            # reduce-scatter this q-block (gpsimd)
            nc.gpsimd.collective_compute(
                "ReduceScatter", ALU.add, replica_groups=GROUPS,
                ins=[part[qi][:, :]], outs=[rs[qi][:, :]])

        # ---------------- main schedule ----------------
        def chain(ci):
            f1, f2 = ffn_units(ci)
            return f1 + f2

        emit_qk(0)
        emit_v(0)
        emit_v(2)
        attn_block(0)
        pending.extend(chain(0))
        emit_qk(1)
        emit_v(4)
        emit_v(6)
        attn_block(1)
        pending.extend(chain(1))
        emit_qk(2)
        emit_v(8)
        emit_v(10)
        pump_box["budget"] = 32   # all of c0 behind the W2 stream
        attn_block(2, pump_every=1, pump_n=1)
        pending.extend(chain(2))
        wload(len(wload_plan))
        emit_qk(3)
        emit_v(12)
        emit_v(14)
        pump_box["budget"] = 30   # leave c2 FFN2 to grind during RS3
        attn_block(3, pump_every=1, pump_n=1)
        pending.extend(chain(3))
        pump_box["budget"] = 10 ** 9
        pump(len(pending))


def _layernorm(nc, pool, acc, y0, uid, a_bcast, b_bcast, eng=None,
               apply_scale=True):
    """Torch-style LayerNorm over the free dim of acc [128, D] f32, in place.
    rsqrt via 2-step Newton (seed y0) -- keeps the Act table on Exp. `eng`
    (DVE or gpsimd) runs the Newton chain + apply; stats are DVE-only."""
    if eng is None:
        eng = nc.vector
    stats = pool.tile([128, 2, 6], F32, tag="lnstats", name=f"lnst{uid}")
    nc.vector.bn_stats(out=stats[:, 0, :], in_=acc[:, 0:512])
    nc.vector.bn_stats(out=stats[:, 1, :], in_=acc[:, 512:1024])
    mv = pool.tile([128, 2], F32, tag="lnmv", name=f"lnmv{uid}")
    nc.vector.bn_aggr(out=mv, in_=stats)
    if not apply_scale:
        # mean-subtract only: relu(y*z) = y*relu(z) and LN2 is invariant to
        # per-row positive scale and constant shift, so the rsqrt never needs
        # computing -- the downstream LN2 absorbs it.
        eng.tensor_scalar_sub(out=acc, in0=acc, scalar1=mv[:, 0:1])
        return
    y = pool.tile([128, 1], F32, tag="lny", name=f"lny{uid}")
    eng.memset(y, y0)
    t_ = pool.tile([128, 1], F32, tag="lnt", name=f"lnt{uid}")
    for _ in range(2):
        eng.tensor_mul(out=t_, in0=y, in1=y)
        eng.tensor_scalar(out=t_, in0=t_, scalar1=mv[:, 1:2],
                          scalar2=-0.5 * CDOF, op0=ALU.mult, op1=ALU.mult)
        eng.tensor_scalar(out=y, in0=t_, scalar1=1.5, scalar2=y,
                          op0=ALU.add, op1=ALU.mult)
    eng.tensor_scalar(out=acc, in0=acc, scalar1=mv[:, 0:1],
                      scalar2=y, op0=ALU.subtract, op1=ALU.mult)
    if a_bcast is not None:
        nc.gpsimd.tensor_mul(out=acc, in0=acc, in1=a_bcast)
    if b_bcast is not None:
        nc.gpsimd.tensor_add(out=acc, in0=acc, in1=b_bcast)


# ======================= host-side runner =======================

_RUNNERS = {}


def _make_runner(nc):
    import jax
    from jax.sharding import Mesh, PartitionSpec, NamedSharding
    import warnings
    with warnings.catch_warnings():
        warnings.simplefilter("ignore")
        from jax.experimental.shard_map import shard_map

    install_neuronx_cc_hook()
    partition_name = (nc.partition_id_tensor.name
                      if nc.partition_id_tensor else None)
    in_names, out_names, out_avals, zero_outs = [], [], [], []
    for alloc in nc.m.functions[0].allocations:
        if not isinstance(alloc, mybir.MemoryLocationSet):
            continue
        name = alloc.memorylocations[0].name
        if alloc.kind == "ExternalInput":
            if name != partition_name:
                in_names.append(name)
        elif alloc.kind == "ExternalOutput":
            out_names.append(name)
            shape = tuple(alloc.tensor_shape)
            dtype = mybir.dt.np(alloc.dtype)
            out_avals.append(jax.core.ShapedArray(shape, dtype))
            zero_outs.append(np.zeros(shape, dtype))
    n_params = len(in_names)
    all_in = list(in_names) + list(out_names)
    if partition_name is not None:
        all_in.append(partition_name)

    def _body(*args):
        operands = list(args)
        if partition_name is not None:
            operands.append(bass2jax.partition_id_tensor())
        outs = _bass_exec_p.bind(
            *operands, out_avals=tuple(out_avals), in_names=tuple(all_in),
            out_names=tuple(out_names), lowering_input_output_aliases=(),
            sim_require_finite=True, sim_require_nnan=True, nc=nc)
        return tuple(outs)

    devices = jax.devices()[:N_CORES]
    mesh = Mesh(np.asarray(devices), ("core",))
    n_outs = len(out_names)
    sharded = jax.jit(
        shard_map(_body, mesh=mesh,
                  in_specs=(PartitionSpec("core"),) * (n_params + n_outs),
                  out_specs=(PartitionSpec("core"),) * n_outs,
                  check_rep=False),
        keep_unused=True)
    sh = NamedSharding(mesh, PartitionSpec("core"))

    def run(in_maps):
        import jax
        concat_in = [np.concatenate([np.asarray(in_maps[c][n])
                                     for c in range(N_CORES)], axis=0)
                     for n in in_names]
        dev_in = [jax.device_put(x, sh) for x in concat_in]
        dev_zero = [jax.device_put(
            np.zeros((N_CORES * z.shape[0], *z.shape[1:]), z.dtype), sh)
            for z in zero_outs]
        outs = sharded(*dev_in, *dev_zero)
        jax.block_until_ready(outs)
        return [
            {name: np.asarray(outs[i]).reshape(N_CORES, *out_avals[i].shape)[c]
             for i, name in enumerate(out_names)}
            for c in range(N_CORES)]

    def run_device(dev_in_and_zeros):
        outs = sharded(*dev_in_and_zeros)
        import jax
        jax.block_until_ready(outs)
        return outs

    run.in_names = in_names
    run.out_names = out_names
    run.zero_outs = zero_outs
    run.sharding = sh
    run.run_device = run_device
    return run


def _prep_inputs(inputs):
    """Shard + pretranspose + quantize the full inputs into 8 per-core maps."""
    x = np.asarray(inputs["x"], np.float32)
    Wqkv = np.asarray(inputs["Wqkv"], np.float32)
    bqkv = np.asarray(inputs["bqkv"], np.float32)
    Wo = np.asarray(inputs["Wo"], np.float32)
    bo = np.asarray(inputs["bo"], np.float32)
    W1 = np.asarray(inputs["W1"], np.float32)
    b1 = np.asarray(inputs["b1"], np.float32)
    W2 = np.asarray(inputs["W2"], np.float32)
    b2 = np.asarray(inputs["b2"], np.float32)
    ln1_a = np.asarray(inputs["ln1_a"], np.float32)
    ln1_b = np.asarray(inputs["ln1_b"], np.float32)
    ln2_a = np.asarray(inputs["ln2_a"], np.float32)
    ln2_b = np.asarray(inputs["ln2_b"], np.float32)

    with_bias = bool(bqkv.any() or bo.any() or b1.any() or b2.any())
    with_affine = bool((ln1_a != 1).any() or ln1_b.any()
                       or (ln2_a != 1).any() or ln2_b.any())

    W1T = W1.T
    W1p = np.ascontiguousarray(
        W1T.reshape(8, 128, 32, 128).transpose(2, 1, 0, 3).reshape(
            32, 128, 1024)).astype(BF)
    W2p = np.ascontiguousarray(W2.T.reshape(32, 128, 1024)).astype(BF)
    mask_tril = np.triu(np.ones((128, 128))).astype(BF)  # keep k <= q

    in_maps = []
    for c in range(N_CORES):
        g, r = divmod(c, TP)
        # Q/K feature permutation: chunk a, partition 32h+dlow
        qfeat = np.empty((2, 128), np.int64)
        for a in range(2):
            for h in range(HPC):
                qfeat[a, 32 * h:32 * h + 32] = (64 * (4 * r + h) + 32 * a
                                                + np.arange(32))
        vfeat = 2 * D + 256 * r + np.arange(256)  # natural (h, dh) order
        wq8 = np.empty((128, 8, 6, 128), np.float32)
        for k in range(8):
            blk = Wqkv[:, k * 128:(k + 1) * 128]  # [3D, 128 dpart]
            for a in range(2):
                wq8[:, k, 0 + a, :] = blk[qfeat[a], :].T
                wq8[:, k, 2 + a, :] = blk[D + qfeat[a], :].T
            wq8[:, k, 4, :] = blk[vfeat[:128], :].T
            wq8[:, k, 5, :] = blk[vfeat[128:], :].T
        wo8 = np.empty((128, 2, D), np.float32)
        for hp in range(2):
            rows = 64 * (4 * r + 2 * hp) + np.arange(128)
            wo8[:, hp, :] = Wo[:, rows].T
        m = {
            "xT8": np.ascontiguousarray(x[g].T).astype(NF8),
            "xsb": np.ascontiguousarray(PART_SCALE * x[g][rows_of(r), :]).astype(BF),
            "wqkv8": (wq8 * SW).astype(NF8),
            "wo8": (wo8 * SW).astype(NF8),
            "w1p": W1p,
            "w2p": W2p,
            "maskt": mask_tril,
        }
        if with_bias:
            bq = np.stack([4.0 * bqkv[qfeat[0]], 4.0 * bqkv[qfeat[1]],
                           4.0 * bqkv[D + qfeat[0]], 4.0 * bqkv[D + qfeat[1]]],
                          axis=1)
            m["bqk"] = np.ascontiguousarray(bq)
            m["bv"] = np.ascontiguousarray(4.0 * bqkv[vfeat])
            m["b1s"] = np.ascontiguousarray(b1.reshape(32, 128).T)
            m["bo_b2"] = np.stack([PART_SCALE * bo, b2])
        if with_affine:
            m["lnab"] = np.stack([ln1_a, ln1_b, ln2_a, ln2_b])
        in_maps.append(m)
    return in_maps, with_bias, with_affine


def get_runner(with_bias=False, with_affine=False):
    key = (with_bias, with_affine)
    if key not in _RUNNERS:
        nc = _build(with_bias, with_affine)
        _RUNNERS[key] = _make_runner(nc)
    return _RUNNERS[key]


def kernel(**inputs) -> np.ndarray:
    in_maps, with_bias, with_affine = _prep_inputs(inputs)
    runner = get_runner(with_bias, with_affine)
    results = runner(in_maps)
    out = np.empty((B, L, D), np.float32)
    for c in range(N_CORES):
        g, r = divmod(c, TP)
        out[g, rows_of(r), :] = results[c]["out"]
    return out



# revision 81
# speedup vs baseline: 1.0291x; 1.0291x over previous
"""Trainium2 Bass kernel for nn_DecoderLayer_15642270892252 (v2, fp8).

Strategy (8 NeuronCores): 2 data-parallel groups over batch B=2; within each
group, 4-way tensor parallel over the 16 heads (4 per core). Attention runs
entirely in fp8e4m3 with DoubleRow matmuls (2 contraction chunks per
instruction at 2x rate): QKV projections, scores (dh=64 split as 2x32), AV
(flipped to [q, dh] orientation with a ones-augmented V column so softmax
denominators fall out of the same matmul), and O-proj. The FFN stays bf16
(fp8 there costs ~1.9e-2 rel err, over the 2e-2 budget); W1/W2 are
SBUF-resident so the 4 chunked FFN passes don't re-stream them.

The O-proj partial sums reduce-scatter in 4 chunks (one per 512-token
q-block; each rank owns 128 rows per chunk), all issued from gpsimd; each
chunk's LN1 + FFN pipeline is emitted interleaved into the remaining
attention blocks (pump()) so the PE stays busy while Act grinds exp.
LayerNorm uses bn_stats + 2-step Newton rsqrt (no Act table thrash: Act
only ever runs Exp/Copy/Relu, which share one activation table); the
tail chunks' LN chains run on DVE (idle there) and the late blocks'
O-proj drains on Act (exp stream finished) to shorten the RS3->FFN tail.

Scales: weights x64 in fp8, activations x4 (Q,K,V,O), exp scale folds
1/(sqrt(dh)*16). The RS wire is fp8: O-proj drains fold 64/256 so the
partials cross at 64x, and the host pre-scales the residual x by 64 --
LN1 is scale-invariant, so the spine normalizes it away (Newton seed
scaled to match). hch/h2 spine and the output are bf16 (host upcasts to
f32). Weight streams are paced one DMA per attention unit (wload), with
wload_until guards keeping pumped FFN matmuls behind their slices. PSUM
rotation: scores/proj "mm" x2, AV "av" x2, FFN2 "acc" x2 banks.
Chunks 0/3's hT transposes run on the PE (identity matmul) -- c0 to
dodge a scheduler-inserted SP-queue wait on the second collective, c3
because the PE is idle on the tail; block 3's pump budget (30) leaves
c2's FFN2 to grind during RS3. In the bias/affine-free case LN1 only
subtracts the mean: relu(y*z)=y*relu(z) and LN2 is invariant to
per-row positive scale, so the rsqrt is never computed (LN2's Newton
seed absorbs the 64x).
"""

import numpy as np
import ml_dtypes

import concourse.bass as bass
import concourse.mybir as mybir
import concourse.tile as tile
from concourse import bacc
from concourse import bass2jax
from concourse.bass2jax import _bass_exec_p, install_neuronx_cc_hook
from concourse.masks import make_identity

F32 = mybir.dt.float32
BF16 = mybir.dt.bfloat16
F8 = mybir.dt.float8e4
AF = mybir.ActivationFunctionType
DR = mybir.MatmulPerfMode.DoubleRow
ALU = mybir.AluOpType
BF = ml_dtypes.bfloat16
NF8 = ml_dtypes.float8_e4m3

B, L, D, H, DH, DFF = 2, 2048, 1024, 16, 64, 4096
EPS = 1e-6
N_CORES = 8
TP = 4
SHARD = L // TP             # 512 rows per rank
HPC = H // TP               # 4 heads per core
GROUPS = [[0, 1, 2, 3], [4, 5, 6, 7]]
NQT = L // 512              # 4 q-blocks == 4 RS chunks
NKT = L // 128              # 16 k-tiles

SW = 64.0                   # weight scale in fp8
DRAIN_QKV = 1.0 / 16.0      # psum(64*Q) -> 4*Q
SCL_EXP = 0.125 / 16.0      # 1/sqrt(dh) / (4*4)
PART_SCALE = 64.0           # fp8 RS wire scale (LN1 is scale-invariant)
DRAIN_PART = PART_SCALE / 256.0   # psum(4*64*part) -> 64*part (fp8 wire)
Y0_LN1 = 1.0 / PART_SCALE   # Newton rsqrt seeds (~1/sqrt(median var));
Y0_LN2 = 0.865              # LN1 input is PART_SCALE*x + wire partials
CDOF = float(D) / float(D - 1)


def rows_of(r):
    """Global L-rows owned by TP rank r (chunk-major: 128 rows per q-block)."""
    return np.concatenate([np.arange(512 * ci + 128 * r, 512 * ci + 128 * (r + 1))
                           for ci in range(NQT)])


def _build(with_bias, with_affine):
    nc = bacc.Bacc()

    xT8 = nc.dram_tensor("xT8", [D, L], F8, kind="ExternalInput")
    xsb = nc.dram_tensor("xsb", [SHARD, D], BF16, kind="ExternalInput")
    wqkv8 = nc.dram_tensor("wqkv8", [128, 8, 6, 128], F8, kind="ExternalInput")
    wo8 = nc.dram_tensor("wo8", [128, 2, D], F8, kind="ExternalInput")
    w1p = nc.dram_tensor("w1p", [32, 128, D], BF16, kind="ExternalInput")
    w2p = nc.dram_tensor("w2p", [32, 128, D], BF16, kind="ExternalInput")
    maskt = nc.dram_tensor("maskt", [128, 128], BF16, kind="ExternalInput")
    if with_bias:
        bqk = nc.dram_tensor("bqk", [128, 4], F32, kind="ExternalInput")
        bv = nc.dram_tensor("bv", [256], F32, kind="ExternalInput")
        b1s = nc.dram_tensor("b1s", [128, 32], F32, kind="ExternalInput")
        bo_b2 = nc.dram_tensor("bo_b2", [2, D], F32, kind="ExternalInput")
    if with_affine:
        lnab = nc.dram_tensor("lnab", [4, D], F32, kind="ExternalInput")

    out = nc.dram_tensor("out", [SHARD, D], BF16, kind="ExternalOutput")

    part = [nc.dram_tensor(f"part{ci}", [512, D], F8) for ci in range(NQT)]
    rs = [nc.dram_tensor(f"rs{ci}", [128, D], F8) for ci in range(NQT)]

    with tile.TileContext(nc) as tc:
        _emit(nc, tc, locals(), with_bias, with_affine)
    nc.finalize()
    return nc


def _emit(nc, tc, t, with_bias, with_affine):
    xT8, xsb, wqkv8, wo8, w1p, w2p, maskt = (
        t["xT8"], t["xsb"], t["wqkv8"], t["wo8"], t["w1p"], t["w2p"], t["maskt"])
    part, rs, out = t["part"], t["rs"], t["out"]

    with tc.tile_pool(name="persist", bufs=1) as P, \
         tc.tile_pool(name="trans", bufs=2) as TR, \
         tc.tile_pool(name="ps", bufs=1, space="PSUM") as PS:

        # ---------------- resident SBUF ----------------
        wqkv_sb = P.tile([128, 8, 6, 128], F8)
        nc.sync.dma_start(out=wqkv_sb, in_=wqkv8[:, :, :, :])
        xT_sb = P.tile([128, 8, L], F8)
        for k in range(8):
            nc.sync.dma_start(out=xT_sb[:, k, :], in_=xT8[k * 128:(k + 1) * 128, :])
        wo_sb = P.tile([128, 2, D], F8)
        nc.sync.dma_start(out=wo_sb, in_=wo8[:, :, :])
        maskt_sb = P.tile([128, 128], BF16)
        nc.sync.dma_start(out=maskt_sb, in_=maskt[:, :])

        qT8 = P.tile([128, 2, L], F8)       # partition = 32*h + dlow
        kT8 = P.tile([128, 2, L], F8)
        v8 = P.tile([128, NKT, HPC, 65], F8)
        w1_sb = P.tile([128, 32, D], BF16)
        w2_sb = P.tile([128, 32, D], BF16)

        identf = P.tile([128, 128], F32)
        make_identity(nc, identf)
        identb = P.tile([128, 128], BF16)
        nc.vector.tensor_copy(out=identb, in_=identf)
        nc.gpsimd.memset(v8[:, :, :, 64:65], 1.0)   # ones-augmented V column

        if with_bias:
            bqk_sb = P.tile([128, 4], F32)
            nc.sync.dma_start(out=bqk_sb, in_=t["bqk"][:, :])
            bv_sb = P.tile([128, HPC, 64], F32)
            nc.sync.dma_start(out=bv_sb,
                              in_=t["bv"][None, :].partition_broadcast(128))
            b1_sb = P.tile([128, 32], F32)
            nc.sync.dma_start(out=b1_sb, in_=t["b1s"][:, :])
            bo_sb = P.tile([128, D], F32)
            nc.sync.dma_start(out=bo_sb, in_=t["bo_b2"][0].partition_broadcast(128))
            b2_sb = P.tile([128, D], F32)
            nc.sync.dma_start(out=b2_sb, in_=t["bo_b2"][1].partition_broadcast(128))
        if with_affine:
            ln_sb = P.tile([128, 4, D], F32)
            nc.sync.dma_start(out=ln_sb,
                              in_=t["lnab"][None, :, :].partition_broadcast(128))

        # W1/W2 resident loads are staggered between attention blocks (SP)
        # so they never starve the part-staging DMAs; see wload() calls below.
        wload_plan = ([("w1", mk) for mk in range(32)]
                      + [("w2", mk) for mk in range(32)])

        def wload(k):
            for _ in range(k):
                if not wload_plan:
                    return
                which, mk = wload_plan.pop(0)
                dst = w1_sb if which == "w1" else w2_sb
                src = w1p if which == "w1" else w2p
                nc.sync.dma_start(out=dst[:, mk, :], in_=src[mk])

        def wload_until(n):
            wload(n - (64 - len(wload_plan)))

        # ---------------- FFN chunk pipeline (pumped closures) ----------------
        boxes = [dict() for _ in range(NQT)]

        def ffn_units(ci):
            bx = boxes[ci]
            units_f1 = []
            units = units_f1

            def u_lnprep():
                # Act-issued: pump pacing places these after the rs chunk has
                # landed so Act doesn't in-order-stall its exp queue.
                rt = TR.tile([128, D], F8, tag="rt", bufs=1, name=f"rt{ci}")
                nc.scalar.dma_start(out=rt, in_=rs[ci][:, :])
                xst = TR.tile([128, D], BF16, tag="xst", bufs=1, name=f"xst{ci}")
                nc.scalar.dma_start(out=xst, in_=xsb[ci * 128:(ci + 1) * 128, :])
                hch = P.tile([128, D], BF16, tag="hch", bufs=2, name=f"hch{ci}")
                bx["h"] = hch
                nc.vector.tensor_add(out=hch, in0=rt, in1=xst)
                if with_bias:
                    nc.vector.tensor_add(out=hch, in0=hch, in1=bo_sb)
                eng = nc.gpsimd if ci in (2,) else nc.vector
                _layernorm(nc, TR, hch, Y0_LN1, ci,
                           ln_sb[:, 0, :] if with_affine else None,
                           ln_sb[:, 1, :] if with_affine else None, eng=eng,
                           apply_scale=with_bias or with_affine)
                if ci in (0, 3):
                    # PE-transpose chunks read hch directly (engine reads are
                    # sem-tracked; the DMA-transpose path needs the snapshot)
                    bx["hb"] = hch
                else:
                    hb = TR.tile([128, D], BF16, tag="hb", bufs=1,
                                 name=f"hb{ci}")
                    eng.tensor_copy(out=hb, in_=hch)
                    bx["hb"] = hb
                bx["hT"] = TR.tile([128, 8, 128], BF16, tag="hT", bufs=1, name=f"hT{ci}")
            units.append(u_lnprep)

            for j0 in range(0, 8, 4):
                def u_tr(j0=j0):
                    hb, hT = bx["hb"], bx["hT"]
                    if ci in (0, 3):
                        # PE transpose: dodges an SP-queue scheduler wait that
                        # parks chunk 0's DMA transposes behind RS1.
                        for j in range(j0, j0 + 4):
                            pt = PS.tile([128, 128], BF16, tag="mm", bufs=2,
                                         name=f"htp{ci}_{j}")
                            nc.tensor.transpose(
                                pt[:, :], hb[:, j * 128:(j + 1) * 128], identb)
                            nc.vector.tensor_copy(out=hT[:, j, :], in_=pt)
                    else:
                        for j in range(j0, j0 + 4):
                            nc.sync.dma_start_transpose(
                                out=hT[:, j, :], in_=hb[:, j * 128:(j + 1) * 128])
                units.append(u_tr)

            def u_f1alloc():
                bx["f1"] = P.tile([128, 32, 128], BF16, tag="f1", bufs=2,
                                  name=f"f1_{ci}")
            units.append(u_f1alloc)

            for mq in range(8):
                def u_f1(mq=mq):
                    wload_until(4 * (mq + 1))
                    hT, f1 = bx["hT"], bx["f1"]
                    psf = PS.tile([128, 4, 128], F32, tag="mm", bufs=2,
                                  name=f"psf{ci}_{mq}")
                    for mm in range(4):
                        m = mq * 4 + mm
                        for k in range(8):
                            nc.tensor.matmul(
                                psf[:, mm, :], w1_sb[:, m, k * 128:(k + 1) * 128],
                                hT[:, k, :], start=(k == 0), stop=(k == 7))
                    if with_bias:
                        for mm in range(4):
                            m = mq * 4 + mm
                            nc.vector.tensor_scalar(
                                out=f1[:, m, :], in0=psf[:, mm, :],
                                scalar1=b1_sb[:, m:m + 1], scalar2=0.0,
                                op0=ALU.add, op1=ALU.max)
                    elif mq % 2:
                        nc.vector.tensor_scalar_max(
                            out=f1[:, mq * 4:(mq + 1) * 4, :], in0=psf,
                            scalar1=0.0)
                    else:
                        nc.scalar.activation(
                            out=f1[:, mq * 4:(mq + 1) * 4, :], in_=psf,
                            func=AF.Relu)
                units.append(u_f1)

            units_f2 = []
            units = units_f2

            for n in range(2):
                for kq in range(8):
                    def u_f2(n=n, kq=kq):
                        wload_until(32 + 4 * (kq + 1))
                        if kq == 0:
                            bx[f"acc{n}"] = PS.tile([128, 512], F32, tag="acc", bufs=2,
                                                    name=f"acc{ci}_{n}")
                        acc, f1 = bx[f"acc{n}"], bx["f1"]
                        for kk in range(4):
                            k = kq * 4 + kk
                            nc.tensor.matmul(
                                acc[:, :], f1[:, k, :],
                                w2_sb[:, k, n * 512:(n + 1) * 512],
                                start=(k == 0), stop=(k == 31))
                    units.append(u_f2)

                def u_f2drain(n=n):
                    acc, hch = bx[f"acc{n}"], bx["h"]
                    nc.vector.tensor_add(out=hch[:, n * 512:(n + 1) * 512],
                                         in0=acc,
                                         in1=hch[:, n * 512:(n + 1) * 512])
                units.append(u_f2drain)

            def u_out():
                hch = bx["h"]
                if with_bias:
                    nc.vector.tensor_add(out=hch, in0=hch, in1=b2_sb)
                y2 = Y0_LN2 if (with_bias or with_affine) \
                    else Y0_LN2 / PART_SCALE
                _layernorm(nc, TR, hch, y2, 4 + ci,
                           ln_sb[:, 2, :] if with_affine else None,
                           ln_sb[:, 3, :] if with_affine else None,
                           eng=nc.gpsimd if ci in (1, 2) else nc.vector)
                nc.sync.dma_start(out=out[ci * 128:(ci + 1) * 128, :], in_=hch)
            units.append(u_out)
            return units_f1, units_f2

        pending = []
        pump_box = {"budget": 10 ** 9}

        def pump(k):
            for _ in range(k):
                if not pending or pump_box["budget"] <= 0:
                    return
                pump_box["budget"] -= 1
                pending.pop(0)()

        # ---------------- QKV ----------------
        def emit_qk(n):
            for i, dst in ((0, qT8), (2, kT8)):
                ps = PS.tile([128, 2, 512], F32, tag="mm", bufs=2,
                             name=f"qk{n}_{i}")
                for a in range(2):
                    for pi in range(4):
                        nc.tensor.matmul(
                            ps[:, a, :], wqkv_sb[:, 2 * pi:2 * pi + 2, i + a, :],
                            xT_sb[:, 2 * pi:2 * pi + 2, n * 512:(n + 1) * 512],
                            start=(pi == 0), stop=(pi == 3), perf_mode=DR)
                od = dst[:, :, n * 512:(n + 1) * 512]
                if with_bias:
                    for a in range(2):
                        nc.vector.tensor_scalar(
                            out=od[:, a, :], in0=ps[:, a, :], scalar1=DRAIN_QKV,
                            scalar2=bqk_sb[:, i + a:i + a + 1],
                            op0=ALU.mult, op1=ALU.add)
                else:
                    nc.vector.tensor_scalar_mul(out=od, in0=ps,
                                                scalar1=DRAIN_QKV)

        def emit_v(t0):
            ps = PS.tile([128, 2, 256], F32, tag="mm", bufs=2, name=f"vv{t0}")
            for tt in range(2):
                for pi in range(4):
                    nc.tensor.matmul(
                        ps[:, tt, :],
                        xT_sb[:, 2 * pi:2 * pi + 2,
                              (t0 + tt) * 128:(t0 + tt + 1) * 128],
                        wqkv_sb[:, 2 * pi:2 * pi + 2, 4:6, :],
                        start=(pi == 0), stop=(pi == 3), perf_mode=DR)
            dst = v8[:, t0:t0 + 2, :, 0:64]
            src = ps.rearrange("p t (h e) -> p t h e", h=HPC)
            nc.vector.tensor_scalar_mul(out=dst, in0=src, scalar1=DRAIN_QKV)
            if with_bias:
                for tt in range(2):
                    nc.vector.tensor_add(out=v8[:, t0 + tt, :, 0:64],
                                         in0=v8[:, t0 + tt, :, 0:64],
                                         in1=bv_sb)

        # ---------------- attention block ----------------
        def attn_block(qi, pump_every=0, pump_n=1):
            npj = 2 * qi + 2
            unit = 0
            # obf[s][:, h, :] = normalized O for token-subtile s, head h
            obfs = [TR.tile([128, HPC, 64], BF16, tag="obf", bufs=5,
                            name=f"obf{qi}_{s}") for s in range(4)]

            def sc_part(h):
                nonlocal unit
                p8s = []
                for pj in range(npj):
                    straddle = pj >= 2 * qi
                    t0 = 2 * (pj - 2 * qi) if straddle else 0
                    ps = PS.tile([128, 2, 512], F32, tag="mm", bufs=2,
                                 name=f"sc{qi}_{h}_{pj}")
                    p8 = TR.tile([128, 2, 512], F8, tag="p8", bufs=9,
                                 name=f"p8_{qi}_{h}_{pj}")
                    p8s.append(p8)
                    qlo = 128 * t0 if straddle else 0
                    for m in range(2):
                        j = 2 * pj + m
                        nc.tensor.matmul(
                            ps[:, m, qlo:512],
                            kT8[32 * h:32 * h + 32, :, j * 128:(j + 1) * 128],
                            qT8[32 * h:32 * h + 32, :,
                                qi * 512 + qlo:(qi + 1) * 512],
                            start=True, stop=True, perf_mode=DR,
                            tile_position=(32 * h, 0))
                    nc.scalar.activation(out=p8[:, :, qlo:512],
                                         in_=ps[:, :, qlo:512],
                                         func=AF.Exp, scale=SCL_EXP)
                    if straddle:
                        for m in range(2):
                            s = t0 + m
                            nc.vector.tensor_mul(
                                out=p8[:, m, s * 128:(s + 1) * 128],
                                in0=p8[:, m, s * 128:(s + 1) * 128],
                                in1=maskt_sb)
                    unit += 1
                    wload(1)
                    if pump_every and unit % pump_every == 0:
                        pump(pump_n)
                return p8s

            def av_part(h, p8s):
                # AV: s-sequential accumulation groups in a single-bank tile
                av = PS.tile([128, 4, 128], F32, tag="av", bufs=2,
                             name=f"av{qi}_{h}")
                for s in range(4):
                    pjs = [pj for pj in range(npj)
                           if not (pj >= 2 * qi and s < 2 * (pj - 2 * qi))]
                    for idx, pj in enumerate(pjs):
                        straddle = pj >= 2 * qi
                        t0 = 2 * (pj - 2 * qi) if straddle else 0
                        first, last = (idx == 0), (idx == len(pjs) - 1)
                        if straddle and s == t0:
                            nc.tensor.matmul(
                                av[:, s, 0:65],
                                p8s[pj][:, 0, s * 128:(s + 1) * 128],
                                v8[:, 2 * pj, h, :],
                                start=first, stop=last)
                        else:
                            nc.tensor.matmul(
                                av[:, s, 0:65],
                                p8s[pj][:, :, s * 128:(s + 1) * 128],
                                v8[:, 2 * pj:2 * pj + 2, h, :],
                                start=first, stop=last, perf_mode=DR)
                rec = TR.tile([128, 4], F32, tag="rec", bufs=2,
                              name=f"rec{qi}_{h}")
                nc.vector.reciprocal(out=rec, in_=av[:, :, 64])
                for s in range(4):
                    nc.vector.tensor_scalar_mul(
                        out=obfs[s][:, h, :], in0=av[:, s, 0:64],
                        scalar1=rec[:, s:s + 1])
                wload(1)
                if pump_every:
                    pump(pump_n)

            # Head lookahead: emit later heads' score matmuls ahead of earlier
            # heads' AV so the PE never parks on an exp round-trip.  Bounded
            # by the p8 pool (9 bufs): block 0 can run all-scores-first,
            # block 1 one head ahead.
            la = 0
            p8s_by_h = {}
            for h in range(HPC):
                p8s_by_h[h] = sc_part(h)
                if h - la >= 0:
                    av_part(h - la, p8s_by_h.pop(h - la))
            for h in sorted(p8s_by_h):
                av_part(h, p8s_by_h[h])

            # transpose + O-proj + stage, per token-subtile
            for s in range(4):
                obf = obfs[s]
                oT = TR.tile([128, 2, 128], F8, tag="oT", name=f"oT{qi}_{s}")
                for hp in range(2):
                    pt = PS.tile([128, 128], BF16, tag="mm", bufs=2,
                                 name=f"otp{qi}_{s}_{hp}")
                    nc.tensor.transpose(
                        pt[:, :],
                        obf[:, 2 * hp:2 * hp + 2, :].rearrange(
                            "p h e -> p (h e)"), identb)
                    if qi <= 1:
                        nc.scalar.activation(out=oT[:, hp, :], in_=pt,
                                             func=AF.Copy)
                    else:
                        nc.vector.tensor_copy(out=oT[:, hp, :], in_=pt)
                pso = PS.tile([128, 2, 512], F32, tag="mm", bufs=2,
                              name=f"pso{qi}_{s}")
                for n in range(2):
                    nc.tensor.matmul(pso[:, n, :], oT[:, :, :],
                                     wo_sb[:, :, n * 512:(n + 1) * 512],
                                     start=True, stop=True, perf_mode=DR)
                st = TR.tile([128, D], F8, tag="st", bufs=2,
                             name=f"st{qi}_{s}")
                psof = pso.rearrange("p n q -> p (n q)")
                nc.scalar.activation(out=st, in_=psof, func=AF.Copy,
                                     scale=DRAIN_PART)
                nc.sync.dma_start(
                    out=part[qi][s * 128:(s + 1) * 128, :], in_=st)
                wload(1)
                if pump_every:
                    pump(pump_n)
            # reduce-scatter this q-block (gpsimd)
            nc.gpsimd.collective_compute(
                "ReduceScatter", ALU.add, replica_groups=GROUPS,
                ins=[part[qi][:, :]], outs=[rs[qi][:, :]])

        # ---------------- main schedule ----------------
        def chain(ci):
            f1, f2 = ffn_units(ci)
            return f1 + f2

        emit_qk(0)
        emit_v(0)
        emit_v(2)
        attn_block(0)
        pending.extend(chain(0))
        emit_qk(1)
        emit_v(4)
        emit_v(6)
        attn_block(1)
        pending.extend(chain(1))
        emit_qk(2)
        emit_v(8)
        emit_v(10)
        pump_box["budget"] = 32   # all of c0 behind the W2 stream
        attn_block(2, pump_every=1, pump_n=1)
        pending.extend(chain(2))
        wload(len(wload_plan))
        emit_qk(3)
        emit_v(12)
        emit_v(14)
        pump_box["budget"] = 30   # leave c2 FFN2 to grind during RS3
        attn_block(3, pump_every=1, pump_n=1)
        pending.extend(chain(3))
        pump_box["budget"] = 10 ** 9
        pump(len(pending))


def _layernorm(nc, pool, acc, y0, uid, a_bcast, b_bcast, eng=None,
               apply_scale=True):
    """Torch-style LayerNorm over the free dim of acc [128, D] f32, in place.
    rsqrt via 2-step Newton (seed y0) -- keeps the Act table on Exp. `eng`
    (DVE or gpsimd) runs the Newton chain + apply; stats are DVE-only."""
    if eng is None:
        eng = nc.vector
    stats = pool.tile([128, 2, 6], F32, tag="lnstats", name=f"lnst{uid}")
    nc.vector.bn_stats(out=stats[:, 0, :], in_=acc[:, 0:512])
    nc.vector.bn_stats(out=stats[:, 1, :], in_=acc[:, 512:1024])
    mv = pool.tile([128, 2], F32, tag="lnmv", name=f"lnmv{uid}")
    nc.vector.bn_aggr(out=mv, in_=stats)
    if not apply_scale:
        # mean-subtract only: relu(y*z) = y*relu(z) and LN2 is invariant to
        # per-row positive scale and constant shift, so the rsqrt never needs
        # computing -- the downstream LN2 absorbs it.
        eng.tensor_scalar_sub(out=acc, in0=acc, scalar1=mv[:, 0:1])
        return
    y = pool.tile([128, 1], F32, tag="lny", name=f"lny{uid}")
    eng.memset(y, y0)
    t_ = pool.tile([128, 1], F32, tag="lnt", name=f"lnt{uid}")
    for _ in range(2):
        eng.tensor_mul(out=t_, in0=y, in1=y)
        eng.tensor_scalar(out=t_, in0=t_, scalar1=mv[:, 1:2],
                          scalar2=-0.5 * CDOF, op0=ALU.mult, op1=ALU.mult)
        eng.tensor_scalar(out=y, in0=t_, scalar1=1.5, scalar2=y,
                          op0=ALU.add, op1=ALU.mult)
    eng.tensor_scalar(out=acc, in0=acc, scalar1=mv[:, 0:1],
                      scalar2=y, op0=ALU.subtract, op1=ALU.mult)
    if a_bcast is not None:
        nc.gpsimd.tensor_mul(out=acc, in0=acc, in1=a_bcast)
    if b_bcast is not None:
        nc.gpsimd.tensor_add(out=acc, in0=acc, in1=b_bcast)


# ======================= host-side runner =======================

_RUNNERS = {}


def _make_runner(nc):
    import jax
    from jax.sharding import Mesh, PartitionSpec, NamedSharding
    import warnings
    with warnings.catch_warnings():
        warnings.simplefilter("ignore")
        from jax.experimental.shard_map import shard_map

    install_neuronx_cc_hook()
    partition_name = (nc.partition_id_tensor.name
                      if nc.partition_id_tensor else None)
    in_names, out_names, out_avals, zero_outs = [], [], [], []
    for alloc in nc.m.functions[0].allocations:
        if not isinstance(alloc, mybir.MemoryLocationSet):
            continue
        name = alloc.memorylocations[0].name
        if alloc.kind == "ExternalInput":
            if name != partition_name:
                in_names.append(name)
        elif alloc.kind == "ExternalOutput":
            out_names.append(name)
            shape = tuple(alloc.tensor_shape)
            dtype = mybir.dt.np(alloc.dtype)
            out_avals.append(jax.core.ShapedArray(shape, dtype))
            zero_outs.append(np.zeros(shape, dtype))
    n_params = len(in_names)
    all_in = list(in_names) + list(out_names)
    if partition_name is not None:
        all_in.append(partition_name)

    def _body(*args):
        operands = list(args)
        if partition_name is not None:
            operands.append(bass2jax.partition_id_tensor())
        outs = _bass_exec_p.bind(
            *operands, out_avals=tuple(out_avals), in_names=tuple(all_in),
            out_names=tuple(out_names), lowering_input_output_aliases=(),
            sim_require_finite=True, sim_require_nnan=True, nc=nc)
        return tuple(outs)

    devices = jax.devices()[:N_CORES]
    mesh = Mesh(np.asarray(devices), ("core",))
    n_outs = len(out_names)
    sharded = jax.jit(
        shard_map(_body, mesh=mesh,
                  in_specs=(PartitionSpec("core"),) * (n_params + n_outs),
                  out_specs=(PartitionSpec("core"),) * n_outs,
                  check_rep=False),
        keep_unused=True)
    sh = NamedSharding(mesh, PartitionSpec("core"))

    def run(in_maps):
        import jax
        concat_in = [np.concatenate([np.asarray(in_maps[c][n])
                                     for c in range(N_CORES)], axis=0)
                     for n in in_names]
        dev_in = [jax.device_put(x, sh) for x in concat_in]
        dev_zero = [jax.device_put(
            np.zeros((N_CORES * z.shape[0], *z.shape[1:]), z.dtype), sh)
            for z in zero_outs]
        outs = sharded(*dev_in, *dev_zero)
        jax.block_until_ready(outs)
        return [
            {name: np.asarray(outs[i]).reshape(N_CORES, *out_avals[i].shape)[c]
             for i, name in enumerate(out_names)}
            for c in range(N_CORES)]

    def run_device(dev_in_and_zeros):
        outs = sharded(*dev_in_and_zeros)
        import jax
        jax.block_until_ready(outs)
        return outs

    run.in_names = in_names
    run.out_names = out_names
    run.zero_outs = zero_outs
    run.sharding = sh
    run.run_device = run_device
    return run


def _prep_inputs(inputs):
    """Shard + pretranspose + quantize the full inputs into 8 per-core maps."""
    x = np.asarray(inputs["x"], np.float32)
    Wqkv = np.asarray(inputs["Wqkv"], np.float32)
    bqkv = np.asarray(inputs["bqkv"], np.float32)
    Wo = np.asarray(inputs["Wo"], np.float32)
    bo = np.asarray(inputs["bo"], np.float32)
    W1 = np.asarray(inputs["W1"], np.float32)
    b1 = np.asarray(inputs["b1"], np.float32)
    W2 = np.asarray(inputs["W2"], np.float32)
    b2 = np.asarray(inputs["b2"], np.float32)
    ln1_a = np.asarray(inputs["ln1_a"], np.float32)
    ln1_b = np.asarray(inputs["ln1_b"], np.float32)
    ln2_a = np.asarray(inputs["ln2_a"], np.float32)
    ln2_b = np.asarray(inputs["ln2_b"], np.float32)

    with_bias = bool(bqkv.any() or bo.any() or b1.any() or b2.any())
    with_affine = bool((ln1_a != 1).any() or ln1_b.any()
                       or (ln2_a != 1).any() or ln2_b.any())

    W1T = W1.T
    W1p = np.ascontiguousarray(
        W1T.reshape(8, 128, 32, 128).transpose(2, 1, 0, 3).reshape(
            32, 128, 1024)).astype(BF)
    W2p = np.ascontiguousarray(W2.T.reshape(32, 128, 1024)).astype(BF)
    mask_tril = np.triu(np.ones((128, 128))).astype(BF)  # keep k <= q

    in_maps = []
    for c in range(N_CORES):
        g, r = divmod(c, TP)
        # Q/K feature permutation: chunk a, partition 32h+dlow
        qfeat = np.empty((2, 128), np.int64)
        for a in range(2):
            for h in range(HPC):
                qfeat[a, 32 * h:32 * h + 32] = (64 * (4 * r + h) + 32 * a
                                                + np.arange(32))
        vfeat = 2 * D + 256 * r + np.arange(256)  # natural (h, dh) order
        wq8 = np.empty((128, 8, 6, 128), np.float32)
        for k in range(8):
            blk = Wqkv[:, k * 128:(k + 1) * 128]  # [3D, 128 dpart]
            for a in range(2):
                wq8[:, k, 0 + a, :] = blk[qfeat[a], :].T
                wq8[:, k, 2 + a, :] = blk[D + qfeat[a], :].T
            wq8[:, k, 4, :] = blk[vfeat[:128], :].T
            wq8[:, k, 5, :] = blk[vfeat[128:], :].T
        wo8 = np.empty((128, 2, D), np.float32)
        for hp in range(2):
            rows = 64 * (4 * r + 2 * hp) + np.arange(128)
            wo8[:, hp, :] = Wo[:, rows].T
        m = {
            "xT8": np.ascontiguousarray(x[g].T).astype(NF8),
            "xsb": np.ascontiguousarray(PART_SCALE * x[g][rows_of(r), :]).astype(BF),
            "wqkv8": (wq8 * SW).astype(NF8),
            "wo8": (wo8 * SW).astype(NF8),
            "w1p": W1p,
            "w2p": W2p,
            "maskt": mask_tril,
        }
        if with_bias:
            bq = np.stack([4.0 * bqkv[qfeat[0]], 4.0 * bqkv[qfeat[1]],
                           4.0 * bqkv[D + qfeat[0]], 4.0 * bqkv[D + qfeat[1]]],
                          axis=1)
            m["bqk"] = np.ascontiguousarray(bq)
            m["bv"] = np.ascontiguousarray(4.0 * bqkv[vfeat])
            m["b1s"] = np.ascontiguousarray(b1.reshape(32, 128).T)
            m["bo_b2"] = np.stack([PART_SCALE * bo, b2])
        if with_affine:
            m["lnab"] = np.stack([ln1_a, ln1_b, ln2_a, ln2_b])
        in_maps.append(m)
    return in_maps, with_bias, with_affine


def get_runner(with_bias=False, with_affine=False):
    key = (with_bias, with_affine)
    if key not in _RUNNERS:
        nc = _build(with_bias, with_affine)
        _RUNNERS[key] = _make_runner(nc)
    return _RUNNERS[key]


def kernel(**inputs) -> np.ndarray:
    in_maps, with_bias, with_affine = _prep_inputs(inputs)
    runner = get_runner(with_bias, with_affine)
    results = runner(in_maps)
    out = np.empty((B, L, D), np.float32)
    for c in range(N_CORES):
        g, r = divmod(c, TP)
        out[g, rows_of(r), :] = results[c]["out"]
    return out

